# revision 17
# baseline (speedup 1.0000x reference)
"""Trainium2 Bass kernel for 2D cubic Hermite interpolation (nn_CubicHermite2d).

Math: with x1 = arange(W), x2 = arange(H) (per the problem spec), the whole
op is linear in `signal`:

    result[b, r, q] = sum_{h,w} M2[h, r] * signal[b, h, w] * M1[w, q]

where M1 [W, Nx] / M2 [H, Ny] are 4-banded cubic-Hermite interpolation
matrices built on the host from xs / ys.  Queries are sorted, so greedy
contiguous query groups have source-row bands inside a single 128-row
window -> every output block is ONE K=128 matmul on the PE (no
accumulation, no transposes):

    step 1:  v[wlo][wp, r]  = sig[hlo:+128, wlo:+128].T @ M2[hlo:+128, rs:re]
    step 2:  out[b, rm, q]  = v[wlo][:, rm*128:+128].T @ M1[wlo:+128, qs:qe]

Matmuls default to float16: 1 cyc/row on the PE, FWL fast weight loads, and
half the load bytes; the inputs are O(1) randn so fp16 range is a non-issue
and the only cost is ~2^-12 input rounding.  The OUTPUT is also stored as
fp16 (half the HBM store traffic -- the dominant cost at 16MB/core in f32)
and cast back to f32 on the host; measured rel-err ~1.5e-3 vs the 2e-2
budget.

Signal is prepacked on the host into the exact SBUF layout
[128, NB * NWIN * W] (one row-window per 128-partition block, batches
side by side), so all loads are large contiguous HWDGE DMAs -- no SWDGE
descriptor-generation serialization (which cost ~13us of Q7 time in v1).

PSUM->SBUF copies (DVE + ACT, the only two engines with a PSUM read port)
are the kernel's hard floor (~30us dense for 52 FD=1024 copies), so the
build keeps both engines saturated: one shared 4-buffer PSUM pool (8
banks) hides PE fills behind copies, and each batch's step2 pairs are
interleaved with the NEXT batch's step1 v tiles so the copy stream never
phase-locks.  Output rows j and j+NY/2 share one DRAM row so every store
descriptor is a 4KB run (2KB descriptors only reach ~260 GB/s); the host
un-interleaves.  The final batch stores per half-block to shorten the
trailing drain.

Sharding: data-parallel over batch B=32 across 8 cores (4 batches/core).
"""

import os
import sys

import numpy as np

for _p in ("/root/.axon_site", "/root/.axon_site/_ro/trn_rl_repo",
           "/root/.axon_site/_ro/pypackages", "/opt/trn_rl_repo"):
    if os.path.isdir(_p) and _p not in sys.path:
        sys.path.append(_p)

import concourse.bass as bass
import concourse.mybir as mybir
from concourse import bacc
from concourse.bass_utils import run_bass_kernel_spmd
from concourse.tile import TileContext

# Problem shapes (hardcoded per spec)
B, H, W = 32, 512, 512
NX, NY = 1024, 1024
N_CORES = 8
NB = B // N_CORES  # batches per core

P = 128
F32 = mybir.dt.float32
F16 = mybir.dt.float16
# matmul operand dtype: f16 (1 cyc/row, ~2^-12 input rounding, FWL weight
# loads, half DMA bytes) | f32r | f32 (exact, 4 cyc/row) | bf16
MM_MODE = os.environ.get("CH2D_DT", "f16")
_MM_DTS = {"f16": mybir.dt.float16, "bf16": mybir.dt.bfloat16,
           "f32r": mybir.dt.float32r, "f32": mybir.dt.float32}
# output dtype: f16 halves store traffic (host casts back to f32)
OUT_MODE = os.environ.get("CH2D_OUT_DT", "f16")
OUT_DT = _MM_DTS["f16"] if OUT_MODE == "f16" else F32
# COARSE_COPY: one [128,1024] PSUM->SBUF copy per out tile (2-bank PSUM
# tiles, fewer engine ops) vs two [128,512] copies (finer pipelining)
COARSE_COPY = os.environ.get("CH2D_COARSE", "1") == "1"
# V_COARSE: coarse FD=1024 copies for step1's v tiles
V_COARSE = os.environ.get("CH2D_VCOARSE", "1") == "1"
VPS_BUFS = int(os.environ.get("CH2D_VPS", "2"))
OPS_BUFS = int(os.environ.get("CH2D_OPS",
                              "2" if COARSE_COPY else ("4" if V_COARSE else "6")))
# ~3.4us of dummy matmuls during the load phase flip the PE HAM clock gate
# from 1.2 to 2.4 GHz before the real stream starts; sized so the warmup
# chain ends right as the first signal batch lands (~10.5us)
WARMUP_MMS = int(os.environ.get("CH2D_WARMUP", "10"))
# pair output rows j and j+NY/2 in one DRAM row -> 4KB store descriptors
# (2KB descriptors run at ~260 GB/s, 4KB at ~420 GB/s: descriptor-rate bound)
PAIR_STORE = os.environ.get("CH2D_PAIR", "1") == "1"
TILE_ENG = os.environ.get("CH2D_TILE_ENG", "0") == "1"
OBUF = int(os.environ.get("CH2D_OBUF", "6"))


def _interp_matrix(x0, u):
    """[n, Q] float64 matrix M with (y @ M) == _interp1d(y, x0, slopes, u) of
    the reference (searchsorted bucket, one-sided/averaged Hermite tangents)."""
    x0 = np.asarray(x0, dtype=np.float64)
    n = len(x0)
    q = len(u)
    d = np.diff(x0)  # d[j] = x0[j+1] - x0[j]
    m = np.zeros((n, q), dtype=np.float64)
    idx = np.searchsorted(x0[1:-1], u.astype(np.float64))
    dxq = d[idx]
    t = (u.astype(np.float64) - x0[idx]) / dxq
    t2, t3 = t * t, t * t * t
    h00 = 1.0 - 3.0 * t2 + 2.0 * t3
    h10 = (t - 2.0 * t2 + t3) * dxq   # multiplies m[I]
    h01 = 3.0 * t2 - 2.0 * t3
    h11 = (t3 - t2) * dxq             # multiplies m[I+1]
    for k in range(q):
        i = int(idx[k])
        m[i, k] += h00[k]
        m[i + 1, k] += h01[k]
        c = h10[k]  # m[I]: one-sided at 0, averaged interior
        if i == 0:
            m[1, k] += c / d[0]
            m[0, k] -= c / d[0]
        else:
            m[i + 1, k] += 0.5 * c / d[i]
            m[i, k] += 0.5 * c * (1.0 / d[i - 1] - 1.0 / d[i])
            m[i - 1, k] -= 0.5 * c / d[i - 1]
        c = h11[k]  # m[I+1]
        if i + 1 == n - 1:
            m[n - 1, k] += c / d[n - 2]
            m[n - 2, k] -= c / d[n - 2]
        else:
            m[i + 2, k] += 0.5 * c / d[i + 1]
            m[i + 1, k] += 0.5 * c * (1.0 / d[i] - 1.0 / d[i + 1])
            m[i, k] -= 0.5 * c / d[i]
    return m, idx.astype(np.int64)


def _make_groups(idx, n, max_size=512, bank=512):
    """Greedy contiguous query groups; each group's source rows fit a
    128-row window starting at row_lo.  Groups never cross `bank`-multiples
    in query index (PSUM bank boundary) and keep even sizes where possible
    (fp32r ISA needs even matmul N and 8B-aligned PSUM column offsets).
    Returns ([(q_start, q_end, row_lo)], f32r_ok)."""
    qn = len(idx)
    lo = np.maximum(idx - 1, 0)
    hi = np.minimum(idx + 2, n - 1)
    groups = []
    s = 0
    while s < qn:
        row_lo = int(lo[s])
        e = s
        while e < qn:
            if hi[e] - row_lo + 1 > P:
                break
            if e - s >= max_size:
                break
            if e > s and (e % bank) == 0:
                break
            e += 1
        if e < qn and (e - s) % 2 == 1 and e - s > 1:
            e -= 1  # keep sizes (and hence starts) even for fp32r
        groups.append((s, e, min(row_lo, n - P)))
        s = e
    f32r_ok = all(q % 2 == 0 and (e - q) % 2 == 0 for q, e, _ in groups)
    return groups, f32r_ok


def _build_nc(g1, g2, mm_dt):
    MM_DT = mm_dt
    nc = bacc.Bacc("TRN2", target_bir_lowering=False,
                   name="cubic_hermite2d", num_devices=N_CORES,
                   num_swdge_queues=1)
    wlo1_list = sorted({g[2] for g in g1})  # distinct xs source windows
    wlo2_list = sorted({g[2] for g in g2})  # distinct ys source windows
    nw2 = len(wlo2_list)
    win_of = {h: j for j, h in enumerate(wlo2_list)}

    # sig packed [128, NB, NWIN2, W]: partition = window row
    sig_d = nc.dram_tensor("sigp", [P, NB * nw2 * W], MM_DT, kind="ExternalInput")
    w2_d = nc.dram_tensor("w2p", [P, NY], MM_DT, kind="ExternalInput")
    w1_d = nc.dram_tensor("w1p", [P, NX], MM_DT, kind="ExternalInput")
    out_d = nc.dram_tensor("out", [NB, NY, NX], OUT_DT, kind="ExternalOutput")

    copy_i = 0
    # per-bank halves so PSUM tiles are single-bank
    half1 = [[g for g in g1 if g[1] <= NX // 2], [g for g in g1 if g[0] >= NX // 2]]
    half2 = [[g for g in g2 if g[1] <= NY // 2], [g for g in g2 if g[0] >= NY // 2]]
    assert sum(map(len, half1)) == len(g1) and sum(map(len, half2)) == len(g2)

    with (
        TileContext(nc) as tc,
        tc.tile_pool(name="const", bufs=1) as const_pool,
        tc.tile_pool(name="sig", bufs=1) as sig_pool,
        tc.tile_pool(name="vbuf", bufs=2 * len(wlo1_list)) as v_pool,
        tc.tile_pool(name="obuf", bufs=OBUF) as o_pool,
        tc.tile_pool(name="ps", bufs=VPS_BUFS + OPS_BUFS, space="PSUM") as ps_pool,
    ):
        # HAM warmup: dummy matmuls during the load phase flip the PE clock
        # gate from 1.2 to 2.4 GHz before the real stream starts.
        if WARMUP_MMS:
            warm = const_pool.tile([P, 512], MM_DT, name="warm")
            nc.vector.memset(warm[:], 0)
            wps = ps_pool.tile([P, NX], F32, name="ps")
            for i in range(WARMUP_MMS):
                nc.tensor.matmul(out=wps[:, :512], lhsT=warm[:, :P],
                                 rhs=warm[:, :512], start=True, stop=True)

        # All loads are large contiguous HWDGE DMAs on the sync ring, which
        # drains FIFO: w2 -> sig b0 (per window, first-use order) -> w1 ->
        # b1 -> b2 -> b3 gives the critical path the full HBM bandwidth and
        # lets the first step1 matmuls start before all of b0 has landed.
        w2_s = const_pool.tile([P, NY], MM_DT, name="w2s")
        nc.sync.dma_start(out=w2_s[:], in_=w2_d[:, :])
        sig_s = sig_pool.tile([P, NB * nw2 * W], MM_DT, name="sigt")
        bb = nw2 * W  # per-batch columns
        for j in range(nw2):
            nc.sync.dma_start(out=sig_s[:, j * W:(j + 1) * W],
                              in_=sig_d[:, j * W:(j + 1) * W])
        w1_s = const_pool.tile([P, NX], MM_DT, name="w1s")
        nc.sync.dma_start(out=w1_s[:], in_=w1_d[:, :])
        for b in range(1, NB):
            nc.sync.dma_start(out=sig_s[:, b * bb:(b + 1) * bb],
                              in_=sig_d[:, b * bb:(b + 1) * bb])

        def sig_ap(b, hlo, wlo):
            base = (b * nw2 + win_of[hlo]) * W + wlo
            return sig_s[:, base:base + P]

        eng_time = [0.0, 0.0]  # [DVE, ACT] modeled queue time (ns)

        def copy_out(dst, src, eng=None):
            # split PSUM->SBUF copies between DVE and ACT, greedily balancing
            # measured time: DVE CAST ~1223ns @FD=1024, ACT ACTIVATE ~1113ns
            nonlocal copy_i
            fd = src.free_size()
            cost = [(120 + fd) / 0.96 + 31, (172 + fd) / 1.2 + 116]
            if eng is not None and TILE_ENG:
                e = eng % 2
            else:
                e = 0 if eng_time[0] + cost[0] <= eng_time[1] + cost[1] else 1
            eng_time[e] += cost[e]
            if e == 0:
                nc.vector.tensor_copy(out=dst, in_=src)
            else:
                nc.scalar.copy(out=dst, in_=src)
            copy_i += 1

        tile_i = [0]

        def build_vtile(b, wlo, v_tiles):
            # one v tile: y-interp of sig[b, :, wlo:wlo+128] at all NY queries
            vt = v_pool.tile([P, NY], MM_DT, name="vt")
            tile_i[0] += 1
            vps = ps_pool.tile([P, NY], F32, name="ps")
            for (rs, re, hlo) in g2:
                nc.tensor.matmul(
                    out=vps[:, rs:re],
                    lhsT=sig_ap(b, hlo, wlo),
                    rhs=w2_s[:, rs:re],
                    start=True, stop=True,
                )
            copy_out(vt[:], vps[:], eng=tile_i[0])
            v_tiles[wlo] = vt

        def step2_rblock(b, mi, v_tiles, ot, sub):
            # matmuls for r-block mi of batch b, copied to ot[:, sub*NX:...]
            ops = ps_pool.tile([P, NX], F32, name="ps")
            for (qs, qe, wlo) in g1:
                nc.tensor.matmul(
                    out=ops[:, qs:qe],
                    lhsT=v_tiles[wlo][:, mi * P:(mi + 1) * P],
                    rhs=w1_s[:, qs:qe],
                    start=True, stop=True,
                )
            copy_out(ot[:, sub * NX:(sub + 1) * NX], ops[:], eng=tile_i[0])

        def build_step2_pair(b, mi, v_tiles, split_store=False):
            # r-blocks (mi, mi+4) of batch b share one staging tile; DRAM row
            # j of out_d holds output rows j and j+NY/2 side by side -> every
            # store descriptor is one contiguous 2*NX run (4KB in fp16).
            # split_store: store each half right after its copy (small trailing
            # drain for the final batch).
            nblk = (NY // P) // 2
            ot = o_pool.tile([P, 2 * NX], OUT_DT, name="ot")
            tile_i[0] += 1
            for sub in range(2):
                step2_rblock(b, mi + sub * nblk, v_tiles, ot, sub)
                if split_store:
                    dst = bass.AP(tensor=out_d,
                                  offset=b * NY * NX + mi * P * 2 * NX + sub * NX,
                                  ap=[[2 * NX, P], [1, NX]])
                    nc.sync.dma_start(out=dst, in_=ot[:, sub * NX:(sub + 1) * NX])
            if not split_store:
                dst = bass.AP(tensor=out_d,
                              offset=b * NY * NX + mi * P * 2 * NX,
                              ap=[[2 * NX, P], [1, 2 * NX]])
                nc.sync.dma_start(out=dst, in_=ot[:])

        v_all = {b: {} for b in range(NB)}
        # software pipeline: batch 0's v tiles first, then for each batch
        # alternate one step2 pair (2 out copies + store) with one of the
        # NEXT batch's v tiles, so the PSUM pool and both copy engines see a
        # steady mixed stream instead of phase-locked step1/step2 stretches.
        nv = len(wlo1_list)
        cuts = [int(nv * (u + 1) / 4 + 0.5) for u in range(4)]
        cuts[-1] = nv
        for wlo in wlo1_list:
            build_vtile(0, wlo, v_all[0])
        for b in range(NB):
            last = b + 1 >= NB
            vdone = 0
            for u, mi in enumerate(range(4)):
                build_step2_pair(b, mi, v_all[b], split_store=last)
                if not last:
                    while vdone < cuts[u]:
                        build_vtile(b + 1, wlo1_list[vdone], v_all[b + 1])
                        vdone += 1

    nc.compile()
    return nc


def _prepare(signal, x1, x2, xs, ys):
    """Host-side prep: sorted-order permutations, interp matrices, groups."""
    xs = np.asarray(xs, dtype=np.float32)
    ys = np.asarray(ys, dtype=np.float32)
    perm_x = None
    if np.any(np.diff(xs) < 0):
        perm_x = np.argsort(xs, kind="stable")
        xs = xs[perm_x]
    perm_y = None
    if np.any(np.diff(ys) < 0):
        perm_y = np.argsort(ys, kind="stable")
        ys = ys[perm_y]

    m1, i1 = _interp_matrix(np.asarray(x1, dtype=np.float64), xs)
    m2, i2 = _interp_matrix(np.asarray(x2, dtype=np.float64), ys)
    g1, ok1 = _make_groups(i1, W)
    g2, ok2 = _make_groups(i2, H)

    # pack band blocks: rows = the group's 128-row source window
    w1p = np.zeros((P, NX), dtype=np.float32)
    for (qs, qe, wlo) in g1:
        w1p[:, qs:qe] = m1[wlo:wlo + P, qs:qe]
    w2p = np.zeros((P, NY), dtype=np.float32)
    for (rs, re, hlo) in g2:
        w2p[:, rs:re] = m2[hlo:hlo + P, rs:re]
    return g1, g2, ok1 and ok2, w1p, w2p, perm_x, perm_y


_NC_CACHE = {}


def _run(inputs, trace=False, trace_kwargs=None):
    signal = np.asarray(inputs["signal"], dtype=np.float32)
    g1, g2, f32r_ok, w1p, w2p, perm_x, perm_y = _prepare(
        signal, inputs["x1"], inputs["x2"], inputs["xs"], inputs["ys"])

    mode = MM_MODE
    if mode == "f32r" and not f32r_ok:
        mode = "f32"
    mm_dt = _MM_DTS[mode]
    key = (tuple(g1), tuple(g2), mm_dt)
    nc = _NC_CACHE.get(key)
    if nc is None:
        nc = _build_nc(g1, g2, mm_dt)
        _NC_CACHE[key] = nc

    np_dt = mybir.dt.np(mm_dt)
    w1c, w2c = w1p.astype(np_dt), w2p.astype(np_dt)
    wlo2_list = sorted({g[2] for g in g2})
    sig_cast = signal.astype(np_dt) if np_dt != np.float32 else signal
    # pack [P, NB, NWIN2, W]: sigp[p, b, j, w] = sig[b, wlo2[j] + p, w]
    # (stacked per core below)
    win = np.stack([sig_cast[:, h:h + P, :] for h in wlo2_list], axis=0)
    # win: [NWIN, B, P, W] -> per-core [P, NB, NWIN, W]
    in_maps = []
    for c in range(N_CORES):
        wc = win[:, c * NB:(c + 1) * NB]          # [NWIN, NB, P, W]
        sp = np.ascontiguousarray(wc.transpose(2, 1, 0, 3))  # [P, NB, NWIN, W]
        in_maps.append({
            "sigp": sp.reshape(P, -1),
            "w2p": w2c,
            "w1p": w1c,
        })
    res = run_bass_kernel_spmd(
        nc, in_maps, core_ids=list(range(N_CORES)),
        trace=trace, **(trace_kwargs or {}),
    )
    out = np.concatenate([r["out"] for r in res.results], axis=0)
    if out.dtype != np.float32:
        out = out.astype(np.float32)
    if PAIR_STORE:
        # DRAM row j holds output rows j and j+NY/2 side by side
        arr = out.reshape(B, NY // 2, 2, NX)
        out = np.concatenate([arr[:, :, 0, :], arr[:, :, 1, :]], axis=1)

    # restore original (unsorted) query order if needed
    if perm_y is not None:
        inv = np.empty_like(perm_y)
        inv[perm_y] = np.arange(len(perm_y))
        out = out[:, inv, :]
    if perm_x is not None:
        inv = np.empty_like(perm_x)
        inv[perm_x] = np.arange(len(perm_x))
        out = out[:, :, inv]
    return out, res


def kernel(signal, x1, x2, xs, ys):
    out, _ = _run({"signal": signal, "x1": x1, "x2": x2, "xs": xs, "ys": ys})
    return out
